# revision 2
# baseline (speedup 1.0000x reference)
"""AttentionFlow kernel v2 for 8 TRN2 NeuronCores.

Sharding: data-parallel over batch B=8, one batch element per core, params
replicated. No collectives.

v2 design (per core):
- Host-prearranged dense layouts for every load (no partition-gather
  rearranges); DMAs spread over SP/ACT/Pool queues; phase-1-critical
  data (qcat, ctx^T chunks) dispatched first.
- Phase 1, per 128-context tile, software-pipelined. The C2Q softmax
  runs WITHOUT a max shift (sim = dot + sq is bounded ~|6| here, exp is
  f32-safe), so exp starts right after the sim matmul. The Q2C max is
  recovered after the fact as e = exp(sc)*max_q(p) with exp(sc)
  host-precomputed (exp(sc+max) = exp(sc)*exp(max) and max(exp) =
  exp(max)). 1/se is folded into the transpose as a diag(1/se) rhs so
  the normalize costs no separate pass; u8/m2 fp8 phase-3 operands are
  cast directly from the u-matmul PSUM (no bf16 staging). Q2C h
  accumulates in a single [1,257] PSUM row via e-column-stationary
  matmuls over host-padded ctx|ones (col 256 gives Z).
- Phase 2: h row -> per-partition hz via 3 tiny matmuls; W3 block folded
  into the ctx-block weights with one scalar_tensor_tensor per half.
- Phase 3: output-stationary g^T [f, c]: weight blocks are lhsT,
  ctx2/u8/m2 stream as rhs in N=512 chunks; u/m blocks fp8 DoubleRow
  (0.5 cyc/row); bias b2 is per-partition in this layout so evacuation
  splits ACT (Identity+bias) / DVE (tensor_scalar_add); bf16 output,
  one dense 512KB DMA per f-tile; host transposes back to [C, F] f32.
"""

import numpy as np
import ml_dtypes

import concourse.bass as bass
import concourse.mybir as mybir
import concourse.tile as tile
from concourse import bacc
from concourse.bass_utils import run_bass_kernel_spmd
from concourse.masks import make_identity

B, C, Q, D = 8, 2048, 128, 256
F = 4 * D          # 1024
CT = C // 128      # 16 context tiles
FT = F // 128      # 8 output f-tiles
FP32 = mybir.dt.float32
BF16 = mybir.dt.bfloat16
FP8 = mybir.dt.float8e4
EXP = mybir.ActivationFunctionType.Exp
DR = mybir.MatmulPerfMode.DoubleRow
AX = mybir.AxisListType.X
MULT = mybir.AluOpType.mult
ADD = mybir.AluOpType.add

# gpsimd(Pool) touching PSUM: flip to False if sim/compile rejects it
POOL_PSUM = True

_cached = {}


def build_nc():
    nc = bacc.Bacc(None, target_bir_lowering=False, debug=False)

    qcat_ext = nc.declare_dram_parameter("qcat", [128, 512], BF16,
                                         isOutput=False)
    ctxt2_ext = nc.declare_dram_parameter("ctxt2", [128, 2 * C], BF16,
                                          isOutput=False)
    cnat_ext = nc.declare_dram_parameter("cnat", [128, CT * 257], FP8,
                                         isOutput=False)
    sq_ext = nc.declare_dram_parameter("sqrow", [1, 512], BF16,
                                       isOutput=False)
    esc_ext = nc.declare_dram_parameter("esc", [128, CT], FP32,
                                        isOutput=False)
    w2tb_ext = nc.declare_dram_parameter("w2tb", [128, 4 * F], BF16,
                                         isOutput=False)
    w2t8_ext = nc.declare_dram_parameter("w2t8", [128, 4 * F], FP8,
                                         isOutput=False)
    b2_ext = nc.declare_dram_parameter("b2c", [128, FT], FP32,
                                       isOutput=False)
    out_ext = nc.declare_dram_parameter("out", [F, C], BF16, isOutput=True)

    ctxt2_v = ctxt2_ext[:, :].rearrange("p (h c) -> p h c", h=2)

    with tile.TileContext(nc) as tc:
        with (
            tc.tile_pool(name="persist", bufs=1) as persist,
            tc.tile_pool(name="p1", bufs=3) as p1,
            tc.tile_pool(name="p3", bufs=2) as p3,
        ):
            # ---------------- persistent tiles ----------------
            qcat = persist.tile([128, 512], BF16, name="qcat", tag="qcat")
            sq4_row = persist.tile([1, 512], BF16, name="sq_row", tag="sq_row")
            ctx2 = persist.tile([128, 2, C], BF16, name="ctx2", tag="ctx2")
            cnat = persist.tile([128, CT // 2, 2, 257], FP8, name="cnat", tag="cnat")
            esc_coll = persist.tile([128, CT], FP32, name="esc", tag="esc")
            w2tb = persist.tile([128, 4, F], BF16, name="w2tb", tag="w2tb")
            w2f = persist.tile([128, 2, F], BF16, name="w2f", tag="w2f")
            w2t8 = persist.tile([128, 4, F], FP8, name="w2t8", tag="w2t8")
            b2c = persist.tile([128, FT], FP32, name="b2c", tag="b2c")
            u8 = persist.tile([128, 2, C], FP8, name="u8", tag="u8")
            m2 = persist.tile([128, 2, C], FP8, name="m2", tag="m2")
            ident = persist.tile([128, 128], BF16, name="ident", tag="ident")
            ones_row = persist.tile([1, 128], BF16, name="ones_r", tag="ones_r")
            pm_coll = persist.tile([128, CT], FP32, name="pm_coll", tag="pm_coll")
            e_coll = persist.tile([128, CT], FP8, name="e_coll", tag="e_coll")

            # ---- prologue DMAs: one queue, priority order (critical
            # phase-1 data first so it doesn't share wire bandwidth) ------
            nc.sync.dma_start(qcat[:, 256:512], qcat_ext[:, 256:512])
            nc.sync.dma_start(sq4_row[:], sq_ext[:, :])
            nc.scalar.dma_start(qcat[:, 0:256], qcat_ext[:, 0:256])
            nc.scalar.dma_start(esc_coll[:], esc_ext[:, :])
            for g in range(8):
                gs = slice(g * 256, (g + 1) * 256)
                nc.sync.dma_start(ctx2[:, :, gs], ctxt2_v[:, :, gs])
            nc.scalar.dma_start(cnat[:], cnat_ext[:, :].rearrange(
                "p (i r d) -> p i r d", i=CT // 2, r=2))
            nc.sync.dma_start(w2tb[:], w2tb_ext[:, :].rearrange(
                "p (t f) -> p t f", t=4))
            nc.sync.dma_start(w2t8[:], w2t8_ext[:, :].rearrange(
                "p (t f) -> p t f", t=4))
            nc.sync.dma_start(b2c[:], b2_ext[:, :])

            make_identity(nc, ident[:])
            nc.gpsimd.memset(ones_row[:], 1.0)
            # warm the ACT exp table while DMAs run
            wexp = p1.tile([1, 1], FP32, name="wexp", tag="wexp")
            nc.scalar.activation(wexp[:], ones_row[0:1, 0:1], EXP)

            with tc.tile_pool(name="p1ps", bufs=1, space="PSUM") as p1ps:
                sim_tiles = [None] * (CT // 4)
                aT_tiles = [None] * (CT // 4)
                hrow_ps = p1ps.tile([1, 257], FP32, name="hrow", tag="hrow",
                                    bufs=1)

                def emit_simquad(Q):
                    sim_ps = p1ps.tile([128, 4, 128], FP32, name=f"sim{Q}",
                                       tag="sim", bufs=2)
                    for t in range(4):
                        i = 4 * Q + t
                        cs = slice(i * 128, (i + 1) * 128)
                        for h in range(2):
                            nc.tensor.matmul(
                                sim_ps[:, t], ctx2[:, h, cs],
                                qcat[:, 256 + h * 128:256 + (h + 1) * 128],
                                start=(h == 0 and t == 0), stop=False,
                                skip_group_check=True,
                            )
                    nc.tensor.matmul(sim_ps[:], ones_row[:], sq4_row[:],
                                     start=False, stop=True,
                                     skip_group_check=True)
                    sim_tiles[Q] = sim_ps

                def emit_soft(Q):
                    sim_ps = sim_tiles[Q]
                    p_bf = p1.tile([128, 4, 128], BF16, name=f"p{Q}",
                                   tag="p", bufs=2)
                    nc.scalar.activation(p_bf[:], sim_ps[:], EXP,
                                         bias=0.0, scale=1.0)
                    se = p1.tile([128, 4], FP32, name=f"se{Q}", tag="se",
                                 bufs=2)
                    nc.vector.reduce_sum(se[:], p_bf[:], axis=AX)
                    nc.vector.reduce_max(pm_coll[:, 4 * Q:4 * Q + 4],
                                         p_bf[:], axis=AX)
                    inv = p1.tile([128, 4], FP32, name=f"inv{Q}", tag="inv",
                                  bufs=2)
                    nc.vector.reciprocal(inv[:], se[:])
                    a_bf = p1.tile([128, 4, 128], BF16, name=f"a{Q}",
                                   tag="a", bufs=2)
                    aT_tiles[Q] = p1ps.tile([128, 512], FP32, name=f"aTp{Q}",
                                            tag="aT", bufs=1)
                    for t in range(4):
                        nc.vector.tensor_scalar_mul(a_bf[:, t], p_bf[:, t],
                                                    inv[:, t:t + 1])
                        nc.tensor.matmul(
                            aT_tiles[Q][:, t * 128:(t + 1) * 128],
                            a_bf[:, t], ident[:], start=True, stop=True)

                def emit_pair(Q):
                    # u matmuls + fp8 staging for quad Q (4 tiles)
                    aT_sb = p1.tile([128, 512], BF16, name=f"ats{Q}",
                                    tag="ats", bufs=2)
                    nc.scalar.copy(aT_sb[:], aT_tiles[Q][:])
                    u_ps = p1ps.tile([128, 2, 512], FP32, name=f"u{Q}",
                                     tag="u", bufs=2)
                    for h in range(2):
                        nc.tensor.matmul(
                            u_ps[:, h], qcat[:, h * 128:(h + 1) * 128],
                            aT_sb[:], start=True, stop=True)
                    cs2 = slice(Q * 512, (Q + 1) * 512)
                    nc.scalar.copy(u8[:, :, cs2], u_ps[:])
                    nc.vector.tensor_tensor(m2[:, :, cs2], u_ps[:],
                                            ctx2[:, :, cs2], MULT)

                def emit_eh(Q):
                    i0 = 4 * Q
                    nc.vector.tensor_tensor(e_coll[:, i0:i0 + 4],
                                            esc_coll[:, i0:i0 + 4],
                                            pm_coll[:, i0:i0 + 4], MULT)
                    for i in range(i0, i0 + 4):
                        nc.tensor.matmul(
                            hrow_ps[:], e_coll[:, i:i + 1],
                            cnat[:, i // 2, i % 2],
                            start=(i == 0), stop=(i == CT - 1))

                NQ = CT // 4
                for Q in range(NQ + 2):
                    if Q < NQ:
                        emit_simquad(Q)
                    if 0 <= Q - 1 < NQ:
                        emit_soft(Q - 1)
                    if 0 <= Q - 2 < NQ:
                        emit_pair(Q - 2)
                    if 2 <= Q - 1 <= NQ:
                        emit_eh(Q - 3) if Q >= 3 else None
                if True:
                    emit_eh(NQ - 2)
                    emit_eh(NQ - 1)

                # ------------- phase 2: Q2C normalization + W fold ---------
                hrow_sb = p1.tile([1, 257], BF16, name="hrow_sb", tag="hrs")
                nc.vector.tensor_copy(hrow_sb[:], hrow_ps[:])
                hT_ps = p1ps.tile([128, 3], FP32, name="hT", tag="aT",
                                  bufs=1)
                for h in range(2):
                    nc.tensor.matmul(hT_ps[:, h:h + 1],
                                     hrow_sb[0:1, h * 128:(h + 1) * 128],
                                     ones_row[0:1, 0:1],
                                     start=True, stop=True)
                nc.tensor.matmul(hT_ps[:, 2:3], ones_row[:],
                                 hrow_sb[0:1, 256:257],
                                 start=True, stop=True)
                invZ = p1.tile([128, 1], FP32, name="invZ", tag="invZ")
                nc.vector.reciprocal(invZ[:], hT_ps[:, 2:3])
                hz = p1.tile([128, 2], FP32, name="hz", tag="hz")
                nc.vector.tensor_scalar_mul(hz[:], hT_ps[:, 0:2], invZ[:])
                for h in range(2):
                    nc.vector.scalar_tensor_tensor(
                        w2f[:, h], w2tb[:, 2 + h], hz[:, h:h + 1],
                        w2tb[:, h], MULT, ADD)

            # ---------------- phase 3: g^T = W^T @ mega^T + b2 -------------
            with tc.tile_pool(name="p3ps", bufs=1, space="PSUM") as p3ps:
                for ft in range(FT):
                    fs = slice(ft * 128, (ft + 1) * 128)
                    gt = p3ps.tile([128, 4 * 512], FP32, name=f"gt{ft}",
                                   tag="gt", bufs=2)
                    blocks = [
                        (u8, w2t8[:, 0:2, fs], DR),
                        (m2, w2t8[:, 2:4, fs], DR),
                        (None, w2f[:, 0, fs], None),
                        (None, w2f[:, 1, fs], None),
                    ]
                    for bi, (data, wsl, pm) in enumerate(blocks):
                        for cc in range(4):
                            ccs = slice(cc * 512, (cc + 1) * 512)
                            if data is not None:
                                rhs = data[:, :, ccs]
                            else:
                                rhs = ctx2[:, bi - 2, ccs]
                            nc.tensor.matmul(gt[:, ccs], wsl, rhs,
                                             start=(bi == 0), stop=(bi == 3),
                                             perf_mode=pm)
                    g_sb = p3.tile([128, C], BF16, name=f"g{ft}", tag="g",
                                   bufs=2)
                    nc.scalar.add(g_sb[:, 0:1024], gt[:, 0:1024],
                                  add=b2c[:, ft:ft + 1])
                    nc.vector.tensor_scalar_add(g_sb[:, 1024:2048],
                                                gt[:, 1024:2048],
                                                b2c[:, ft:ft + 1])
                    nc.sync.dma_start(out_ext[fs, 0:1024], g_sb[:, 0:1024])
                    nc.gpsimd.dma_start(out_ext[fs, 1024:2048],
                                        g_sb[:, 1024:2048])

    nc.finalize()
    return nc


def make_in_maps(inputs):
    """Build per-core input maps from full (unsharded) numpy inputs."""
    bf16 = ml_dtypes.bfloat16
    fp8 = ml_dtypes.float8_e4m3fn
    questions = np.asarray(inputs["questions"], dtype=np.float32)
    contexts = np.asarray(inputs["contexts"], dtype=np.float32)
    w_sim = np.asarray(inputs["w_sim"], dtype=np.float32)
    W2 = np.asarray(inputs["W2"], dtype=np.float32)
    w2t = np.ascontiguousarray(W2.T).astype(np.float32)   # [F(mega), F(out)]
    wc, wq, wcq = w_sim[:D], w_sim[D:2 * D], w_sim[2 * D:]

    # w2tb: [128, 4, F] bf16 — t=0,1: W0 halves (ctx block), t=2,3: W3
    # halves (h*ctx block)
    w2tb = np.stack([w2t[0:128], w2t[128:256],
                     w2t[768:896], w2t[896:1024]], axis=1)
    # w2t8: [128, 4, F] fp8 — t=0,1: W1 halves (u block), t=2,3: W2b
    # halves (u*ctx block); DR packing [p, h, f] with d = base + h*128 + p
    w2t8 = np.stack([w2t[256:384], w2t[384:512],
                     w2t[512:640], w2t[640:768]], axis=1)
    w2t8 = np.clip(w2t8, -240, 240).astype(fp8).reshape(128, 4 * F)
    w2tb = w2tb.astype(bf16).reshape(128, 4 * F)
    b2c = np.ascontiguousarray(
        np.asarray(inputs["b2"], dtype=np.float32).reshape(FT, 128).T)

    in_maps = []
    for i in range(B):
        qi = questions[i].astype(bf16)
        ci = contexts[i].astype(bf16)
        cif = ci.astype(np.float32)
        qT = qi.astype(np.float32).T                      # [D, Q]
        qmod = qT * wcq[:, None]                          # [D, Q]
        qcat = np.concatenate(
            [qi.astype(np.float32),
             np.concatenate([qmod[0:128], qmod[128:256]], axis=1)], axis=1)
        sqv = qi.astype(np.float32) @ wq.astype(np.float32)
        # esc = exp(sc), sc = ctx @ wc (bf16 ctx to match device rounding)
        sc = cif @ wc
        esc = (np.exp(sc) / 32.0).reshape(CT, 128).T      # [128, CT]
        # ctxt2: [128, 2, C] with ctxt2[p, h, c] = ci[c, h*128+p]
        ciT = np.ascontiguousarray(ci.T)                  # [D, C] bf16
        ctxt2 = np.ascontiguousarray(
            ciT.reshape(2, 128, C).transpose(1, 0, 2)).reshape(128, 2 * C)
        # cnat: [128, CT//2, 2, 257] fp8 DR-pair-packed:
        # cnat[p, pr, r, :256] = ci[(2*pr+r)*128+p], col 256 = 1
        cnat = np.ones((128, CT // 2, 2, 257), np.float32)
        cnat[:, :, :, 0:256] = contexts[i].reshape(CT // 2, 2, 128,
                                                   256).transpose(2, 0, 1, 3)
        in_maps.append({
            "qcat": qcat.astype(bf16),
            "sqrow": np.tile(sqv.reshape(1, 128), (1, 4)).astype(bf16),
            "esc": np.ascontiguousarray(esc).astype(np.float32),
            "ctxt2": ctxt2,
            "cnat": np.clip(cnat, -240, 240).astype(fp8).reshape(128, CT * 257),
            "w2tb": w2tb,
            "w2t8": w2t8,
            "b2c": b2c,
        })
    return in_maps


def gather_out(res):
    """Stack per-core [F, C] bf16 outputs into [B, C, F] f32."""
    return np.stack(
        [np.asarray(res.results[i]["out"]).astype(np.float32).T
         for i in range(B)], axis=0)


def kernel(questions, contexts, questions_mask, contexts_mask, w_sim, W2, b2):
    if "nc" not in _cached:
        _cached["nc"] = build_nc()
    nc = _cached["nc"]
    in_maps = make_in_maps({
        "questions": questions, "contexts": contexts,
        "w_sim": w_sim, "W2": W2, "b2": b2,
    })
    res = run_bass_kernel_spmd(nc, in_maps, core_ids=list(range(B)))
    return gather_out(res)


# revision 3
# speedup vs baseline: 1.0305x; 1.0305x over previous
"""AttentionFlow kernel v2 for 8 TRN2 NeuronCores.

Sharding: data-parallel over batch B=8, one batch element per core, params
replicated. No collectives.

v2 design (per core):
- Host-prearranged dense layouts for every load (no partition-gather
  rearranges); DMAs spread over SP/ACT/Pool queues; phase-1-critical
  data (qcat, ctx^T chunks) dispatched first.
- Phase 1, per 128-context tile, software-pipelined. The C2Q softmax
  runs WITHOUT a max shift (sim = dot + sq is bounded ~|6| here, exp is
  f32-safe), so exp starts right after the sim matmul. The Q2C max is
  recovered after the fact as e = exp(sc)*max_q(p) with exp(sc)
  host-precomputed (exp(sc+max) = exp(sc)*exp(max) and max(exp) =
  exp(max)). 1/se is folded into the transpose as a diag(1/se) rhs so
  the normalize costs no separate pass; u8/m2 fp8 phase-3 operands are
  cast directly from the u-matmul PSUM (no bf16 staging). Q2C h
  accumulates in a single [1,257] PSUM row via e-column-stationary
  matmuls over host-padded ctx|ones (col 256 gives Z).
- Phase 2: h row -> per-partition hz via 3 tiny matmuls; W3 block folded
  into the ctx-block weights with one scalar_tensor_tensor per half.
- Phase 3: output-stationary g^T [f, c]: weight blocks are lhsT,
  ctx2/u8/m2 stream as rhs in N=512 chunks; u/m blocks fp8 DoubleRow
  (0.5 cyc/row); bias b2 is per-partition in this layout so evacuation
  splits ACT (Identity+bias) / DVE (tensor_scalar_add); bf16 output,
  one dense 512KB DMA per f-tile; host transposes back to [C, F] f32.
"""

import numpy as np
import ml_dtypes

import concourse.bass as bass
import concourse.mybir as mybir
import concourse.tile as tile
from concourse import bacc
from concourse.bass_utils import run_bass_kernel_spmd
from concourse.masks import make_identity

B, C, Q, D = 8, 2048, 128, 256
F = 4 * D          # 1024
CT = C // 128      # 16 context tiles
FT = F // 128      # 8 output f-tiles
FP32 = mybir.dt.float32
BF16 = mybir.dt.bfloat16
FP8 = mybir.dt.float8e4
EXP = mybir.ActivationFunctionType.Exp
DR = mybir.MatmulPerfMode.DoubleRow
AX = mybir.AxisListType.X
MULT = mybir.AluOpType.mult
ADD = mybir.AluOpType.add

# gpsimd(Pool) touching PSUM: flip to False if sim/compile rejects it
POOL_PSUM = True

_cached = {}


def build_nc():
    nc = bacc.Bacc(None, target_bir_lowering=False, debug=False)

    qcat_ext = nc.declare_dram_parameter("qcat", [128, 512], BF16,
                                         isOutput=False)
    ctxt2_ext = nc.declare_dram_parameter("ctxt2", [128, 2 * C], BF16,
                                          isOutput=False)
    cnat_ext = nc.declare_dram_parameter("cnat", [128, CT * 257], FP8,
                                         isOutput=False)
    sq_ext = nc.declare_dram_parameter("sqrow", [1, 512], BF16,
                                       isOutput=False)
    esc_ext = nc.declare_dram_parameter("esc", [128, CT], FP32,
                                        isOutput=False)
    w2tb_ext = nc.declare_dram_parameter("w2tb", [128, 4 * F], BF16,
                                         isOutput=False)
    w2t8_ext = nc.declare_dram_parameter("w2t8", [128, 4 * F], FP8,
                                         isOutput=False)
    b2_ext = nc.declare_dram_parameter("b2c", [128, FT], FP32,
                                       isOutput=False)
    out_ext = nc.declare_dram_parameter("out", [F, C], BF16, isOutput=True)

    ctxt2_v = ctxt2_ext[:, :].rearrange("p (h c) -> p h c", h=2)

    with tile.TileContext(nc) as tc:
        with (
            tc.tile_pool(name="persist", bufs=1) as persist,
            tc.tile_pool(name="p1", bufs=3) as p1,
            tc.tile_pool(name="p3", bufs=2) as p3,
        ):
            # ---------------- persistent tiles ----------------
            qcat = persist.tile([128, 512], BF16, name="qcat", tag="qcat")
            sq4_row = persist.tile([1, 512], BF16, name="sq_row", tag="sq_row")
            ctx2 = persist.tile([128, 2, C], BF16, name="ctx2", tag="ctx2")
            cnat = persist.tile([128, CT // 2, 2, 257], FP8, name="cnat", tag="cnat")
            esc_coll = persist.tile([128, CT], FP32, name="esc", tag="esc")
            w2tb = persist.tile([128, 4, F], BF16, name="w2tb", tag="w2tb")
            w2f = persist.tile([128, 2, F], BF16, name="w2f", tag="w2f")
            w2t8 = persist.tile([128, 4, F], FP8, name="w2t8", tag="w2t8")
            b2c = persist.tile([128, FT], FP32, name="b2c", tag="b2c")
            u8 = persist.tile([128, 2, C], FP8, name="u8", tag="u8")
            m2 = persist.tile([128, 2, C], FP8, name="m2", tag="m2")
            ident = persist.tile([128, 128], BF16, name="ident", tag="ident")
            ones_row = persist.tile([1, 128], BF16, name="ones_r", tag="ones_r")
            pm_coll = persist.tile([128, CT], FP32, name="pm_coll", tag="pm_coll")
            e_coll = persist.tile([128, CT], FP8, name="e_coll", tag="e_coll")

            # ---- prologue DMAs: one queue, priority order (critical
            # phase-1 data first so it doesn't share wire bandwidth) ------
            nc.sync.dma_start(qcat[:, 256:512], qcat_ext[:, 256:512])
            nc.sync.dma_start(sq4_row[:], sq_ext[:, :])
            nc.scalar.dma_start(qcat[:, 0:256], qcat_ext[:, 0:256])
            nc.scalar.dma_start(esc_coll[:], esc_ext[:, :])
            for g in range(8):
                gs = slice(g * 256, (g + 1) * 256)
                nc.sync.dma_start(ctx2[:, :, gs], ctxt2_v[:, :, gs])
            nc.scalar.dma_start(cnat[:], cnat_ext[:, :].rearrange(
                "p (i r d) -> p i r d", i=CT // 2, r=2))
            nc.sync.dma_start(w2tb[:], w2tb_ext[:, :].rearrange(
                "p (t f) -> p t f", t=4))
            nc.sync.dma_start(w2t8[:], w2t8_ext[:, :].rearrange(
                "p (t f) -> p t f", t=4))
            nc.sync.dma_start(b2c[:], b2_ext[:, :])

            make_identity(nc, ident[:])
            nc.gpsimd.memset(ones_row[:], 1.0)
            # warm the ACT exp table while DMAs run
            wexp = p1.tile([1, 1], FP32, name="wexp", tag="wexp")
            nc.scalar.activation(wexp[:], ones_row[0:1, 0:1], EXP)

            with tc.tile_pool(name="p1ps", bufs=1, space="PSUM") as p1ps:
                sim_tiles = [None] * (CT // 4)
                aT_tiles = [None] * (CT // 4)
                hrow_ps = p1ps.tile([1, 257], FP32, name="hrow", tag="hrow",
                                    bufs=1)

                def emit_simquad(Q):
                    sim_ps = p1ps.tile([128, 4, 128], FP32, name=f"sim{Q}",
                                       tag="sim", bufs=2)
                    for t in range(4):
                        i = 4 * Q + t
                        cs = slice(i * 128, (i + 1) * 128)
                        for h in range(2):
                            nc.tensor.matmul(
                                sim_ps[:, t], ctx2[:, h, cs],
                                qcat[:, 256 + h * 128:256 + (h + 1) * 128],
                                start=(h == 0 and t == 0), stop=False,
                                skip_group_check=True,
                            )
                    nc.tensor.matmul(sim_ps[:], ones_row[:], sq4_row[:],
                                     start=False, stop=True,
                                     skip_group_check=True)
                    sim_tiles[Q] = sim_ps

                def emit_soft(Q):
                    sim_ps = sim_tiles[Q]
                    p_bf = p1.tile([128, 4, 128], BF16, name=f"p{Q}",
                                   tag="p", bufs=2)
                    nc.scalar.activation(p_bf[:], sim_ps[:], EXP,
                                         bias=0.0, scale=1.0)
                    se = p1.tile([128, 4], FP32, name=f"se{Q}", tag="se",
                                 bufs=2)
                    nc.vector.reduce_sum(se[:], p_bf[:], axis=AX)
                    nc.vector.reduce_max(pm_coll[:, 4 * Q:4 * Q + 4],
                                         p_bf[:], axis=AX)
                    inv = p1.tile([128, 4], FP32, name=f"inv{Q}", tag="inv",
                                  bufs=2)
                    nc.vector.reciprocal(inv[:], se[:])
                    a_bf = p1.tile([128, 4, 128], BF16, name=f"a{Q}",
                                   tag="a", bufs=2)
                    aT_tiles[Q] = p1ps.tile([128, 512], FP32, name=f"aTp{Q}",
                                            tag="aT", bufs=1)
                    for t in range(4):
                        nc.vector.tensor_scalar_mul(a_bf[:, t], p_bf[:, t],
                                                    inv[:, t:t + 1])
                        nc.tensor.matmul(
                            aT_tiles[Q][:, t * 128:(t + 1) * 128],
                            a_bf[:, t], ident[:], start=True, stop=True)

                def emit_pair(Q):
                    # u matmuls + fp8 staging for quad Q (4 tiles)
                    aT_sb = p1.tile([128, 512], BF16, name=f"ats{Q}",
                                    tag="ats", bufs=2)
                    nc.scalar.copy(aT_sb[:], aT_tiles[Q][:])
                    u_ps = p1ps.tile([128, 2, 512], FP32, name=f"u{Q}",
                                     tag="u", bufs=2)
                    for h in range(2):
                        nc.tensor.matmul(
                            u_ps[:, h], qcat[:, h * 128:(h + 1) * 128],
                            aT_sb[:], start=True, stop=True)
                    cs2 = slice(Q * 512, (Q + 1) * 512)
                    nc.scalar.copy(u8[:, :, cs2], u_ps[:])
                    nc.vector.tensor_tensor(m2[:, :, cs2], u_ps[:],
                                            ctx2[:, :, cs2], MULT)

                def emit_eh(Q):
                    i0 = 4 * Q
                    nc.vector.tensor_tensor(e_coll[:, i0:i0 + 4],
                                            esc_coll[:, i0:i0 + 4],
                                            pm_coll[:, i0:i0 + 4], MULT)
                    for i in range(i0, i0 + 4):
                        nc.tensor.matmul(
                            hrow_ps[:], e_coll[:, i:i + 1],
                            cnat[:, i // 2, i % 2],
                            start=(i == 0), stop=(i == CT - 1))

                NQ = CT // 4
                for Q in range(NQ + 2):
                    if Q < NQ:
                        emit_simquad(Q)
                    if 0 <= Q - 1 < NQ:
                        emit_soft(Q - 1)
                    if 0 <= Q - 2 < NQ:
                        emit_pair(Q - 2)
                    if 2 <= Q - 1 <= NQ:
                        emit_eh(Q - 3) if Q >= 3 else None
                if True:
                    emit_eh(NQ - 2)
                    emit_eh(NQ - 1)

                # ------------- phase 2: Q2C normalization + W fold ---------
                hrow_sb = p1.tile([1, 257], BF16, name="hrow_sb", tag="hrs")
                nc.vector.tensor_copy(hrow_sb[:], hrow_ps[:])
                hT_ps = p1ps.tile([128, 3], FP32, name="hT", tag="aT",
                                  bufs=1)
                for h in range(2):
                    nc.tensor.matmul(hT_ps[:, h:h + 1],
                                     hrow_sb[0:1, h * 128:(h + 1) * 128],
                                     ones_row[0:1, 0:1],
                                     start=True, stop=True)
                nc.tensor.matmul(hT_ps[:, 2:3], ones_row[:],
                                 hrow_sb[0:1, 256:257],
                                 start=True, stop=True)
                invZ = p1.tile([128, 1], FP32, name="invZ", tag="invZ")
                nc.vector.reciprocal(invZ[:], hT_ps[:, 2:3])
                hz = p1.tile([128, 2], FP32, name="hz", tag="hz")
                nc.vector.tensor_scalar_mul(hz[:], hT_ps[:, 0:2], invZ[:])
                for h in range(2):
                    nc.vector.scalar_tensor_tensor(
                        w2f[:, h], w2tb[:, 2 + h], hz[:, h:h + 1],
                        w2tb[:, h], MULT, ADD)

            # ---------------- phase 3: g^T = W^T @ mega^T + b2 -------------
            with tc.tile_pool(name="p3ps", bufs=1, space="PSUM") as p3ps:
                for ft in range(FT):
                    fs = slice(ft * 128, (ft + 1) * 128)
                    gt = p3ps.tile([128, 4 * 512], FP32, name=f"gt{ft}",
                                   tag="gt", bufs=2)
                    blocks = [
                        (u8, w2t8[:, 0:2, fs], DR),
                        (m2, w2t8[:, 2:4, fs], DR),
                        (None, w2f[:, 0, fs], None),
                        (None, w2f[:, 1, fs], None),
                    ]
                    for bi, (data, wsl, pm) in enumerate(blocks):
                        for cc in range(4):
                            ccs = slice(cc * 512, (cc + 1) * 512)
                            if data is not None:
                                rhs = data[:, :, ccs]
                            else:
                                rhs = ctx2[:, bi - 2, ccs]
                            nc.tensor.matmul(gt[:, ccs], wsl, rhs,
                                             start=(bi == 0), stop=(bi == 3),
                                             perf_mode=pm)
                    g_sb = p3.tile([128, C], BF16, name=f"g{ft}", tag="g",
                                   bufs=2)
                    nc.scalar.add(g_sb[:, 0:1024], gt[:, 0:1024],
                                  add=b2c[:, ft:ft + 1])
                    nc.vector.tensor_scalar_add(g_sb[:, 1024:2048],
                                                gt[:, 1024:2048],
                                                b2c[:, ft:ft + 1])
                    nc.sync.dma_start(out_ext[fs, 0:1024], g_sb[:, 0:1024])
                    nc.sync.dma_start(out_ext[fs, 1024:2048],
                                      g_sb[:, 1024:2048])

    nc.finalize()
    return nc


def make_in_maps(inputs):
    """Build per-core input maps from full (unsharded) numpy inputs."""
    bf16 = ml_dtypes.bfloat16
    fp8 = ml_dtypes.float8_e4m3fn
    questions = np.asarray(inputs["questions"], dtype=np.float32)
    contexts = np.asarray(inputs["contexts"], dtype=np.float32)
    w_sim = np.asarray(inputs["w_sim"], dtype=np.float32)
    W2 = np.asarray(inputs["W2"], dtype=np.float32)
    w2t = np.ascontiguousarray(W2.T).astype(np.float32)   # [F(mega), F(out)]
    wc, wq, wcq = w_sim[:D], w_sim[D:2 * D], w_sim[2 * D:]

    # w2tb: [128, 4, F] bf16 — t=0,1: W0 halves (ctx block), t=2,3: W3
    # halves (h*ctx block)
    w2tb = np.stack([w2t[0:128], w2t[128:256],
                     w2t[768:896], w2t[896:1024]], axis=1)
    # w2t8: [128, 4, F] fp8 — t=0,1: W1 halves (u block), t=2,3: W2b
    # halves (u*ctx block); DR packing [p, h, f] with d = base + h*128 + p
    w2t8 = np.stack([w2t[256:384], w2t[384:512],
                     w2t[512:640], w2t[640:768]], axis=1)
    w2t8 = np.clip(w2t8, -240, 240).astype(fp8).reshape(128, 4 * F)
    w2tb = w2tb.astype(bf16).reshape(128, 4 * F)
    b2c = np.ascontiguousarray(
        np.asarray(inputs["b2"], dtype=np.float32).reshape(FT, 128).T)

    in_maps = []
    for i in range(B):
        qi = questions[i].astype(bf16)
        ci = contexts[i].astype(bf16)
        cif = ci.astype(np.float32)
        qT = qi.astype(np.float32).T                      # [D, Q]
        qmod = qT * wcq[:, None]                          # [D, Q]
        qcat = np.concatenate(
            [qi.astype(np.float32),
             np.concatenate([qmod[0:128], qmod[128:256]], axis=1)], axis=1)
        sqv = qi.astype(np.float32) @ wq.astype(np.float32)
        # esc = exp(sc), sc = ctx @ wc (bf16 ctx to match device rounding)
        sc = cif @ wc
        esc = (np.exp(sc) / 32.0).reshape(CT, 128).T      # [128, CT]
        # ctxt2: [128, 2, C] with ctxt2[p, h, c] = ci[c, h*128+p]
        ciT = np.ascontiguousarray(ci.T)                  # [D, C] bf16
        ctxt2 = np.ascontiguousarray(
            ciT.reshape(2, 128, C).transpose(1, 0, 2)).reshape(128, 2 * C)
        # cnat: [128, CT//2, 2, 257] fp8 DR-pair-packed:
        # cnat[p, pr, r, :256] = ci[(2*pr+r)*128+p], col 256 = 1
        cnat = np.ones((128, CT // 2, 2, 257), np.float32)
        cnat[:, :, :, 0:256] = contexts[i].reshape(CT // 2, 2, 128,
                                                   256).transpose(2, 0, 1, 3)
        in_maps.append({
            "qcat": qcat.astype(bf16),
            "sqrow": np.tile(sqv.reshape(1, 128), (1, 4)).astype(bf16),
            "esc": np.ascontiguousarray(esc).astype(np.float32),
            "ctxt2": ctxt2,
            "cnat": np.clip(cnat, -240, 240).astype(fp8).reshape(128, CT * 257),
            "w2tb": w2tb,
            "w2t8": w2t8,
            "b2c": b2c,
        })
    return in_maps


def gather_out(res):
    """Stack per-core [F, C] bf16 outputs into [B, C, F] f32."""
    return np.stack(
        [np.asarray(res.results[i]["out"]).astype(np.float32).T
         for i in range(B)], axis=0)


def kernel(questions, contexts, questions_mask, contexts_mask, w_sim, W2, b2):
    if "nc" not in _cached:
        _cached["nc"] = build_nc()
    nc = _cached["nc"]
    in_maps = make_in_maps({
        "questions": questions, "contexts": contexts,
        "w_sim": w_sim, "W2": W2, "b2": b2,
    })
    res = run_bass_kernel_spmd(nc, in_maps, core_ids=list(range(B)))
    return gather_out(res)
